# revision 1
# baseline (speedup 1.0000x reference)
"""CapsuleLayer (dynamic routing) Trainium2 kernel.

Problem: B=128, I=1152 input capsules (A=8), O=10 output capsules (OA=16),
3 routing iterations.  Data-parallel over batch: 8 cores x 16 examples.

Per-core layout ("P2"): SBUF partition p = is*16 + b  (is = i mod 8, b = local
batch), chunk c = i // 8 in the free dim, vote coordinate n = oa*10 + o
(o innermost so the squash/softmax reductions are innermost-axis reductions).

Phases (bf16 data / f32 accumulation):
  votes:  72 paired-chunk matmuls [k=128=(pair,is,a), m=128=(is,b), n=320]
          with a host-built block-diagonal x as the stationary operand.
  iter t: softmax(logits) -> wv = votes*route (3 pieces, overlapping the
          s-matmuls) -> s via 48 accumulating 480-wide matmuls with a constant
          0/1 selection lhsT (sums over i) -> squash -> v replicated to all
          partitions via a constant matmul -> delta = sum_oa votes*v via
          elementwise mul + in-place pair-tree reduce -> logits += delta.
          The last iteration stops after squash and DMAs v out.
"""

import numpy as np
import ml_dtypes

B, I, A, O, OA = 128, 1152, 8, 10, 16
NCORES = 8
BL = B // NCORES        # 16 examples per core
IS8 = 8                 # i-positions per half-chunk
C = I // IS8            # 144 half-chunks
CP = C // 2             # 72 paired chunks
N = O * OA              # 160, n = oa*O + o
N2 = 2 * N              # 320 per paired chunk
P = 128                 # p = is*BL + b
NUM_ROUTING = 3

# DVE/GPSIMD split of the big elementwise passes (by half-chunk range)
CSPLIT = C  # GPSIMD TT measured ~6x slower than DVE; keep all on DVE

_NC_CACHE = {}


def _build_nc():
    from contextlib import ExitStack

    import concourse.tile as tile
    import concourse.mybir as mybir
    from concourse import bacc

    F32 = mybir.dt.float32
    BF16 = mybir.dt.bfloat16
    AF = mybir.ActivationFunctionType
    ALU = mybir.AluOpType
    AX = mybir.AxisListType

    nc = bacc.Bacc()
    xbd_d = nc.dram_tensor("xbd", [P, CP, P], BF16, kind="ExternalInput")
    w2c_d = nc.dram_tensor("w2c", [P, CP, N2], BF16, kind="ExternalInput")
    bsel_d = nc.dram_tensor("bsel", [P, BL], BF16, kind="ExternalInput")
    brep_d = nc.dram_tensor("brep", [BL, P], BF16, kind="ExternalInput")
    bias_d = nc.dram_tensor("biasr", [BL, N], F32, kind="ExternalInput")
    vout_d = nc.dram_tensor("vout", [BL, N], F32, kind="ExternalOutput")

    with ExitStack() as ctx:
        tc = ctx.enter_context(tile.TileContext(nc))
        st = ctx.enter_context(tc.tile_pool(name="static", bufs=1))
        itp = ctx.enter_context(tc.tile_pool(name="itp", bufs=1))

        w2c = st.tile([P, CP, N2], BF16)
        votes = st.tile([P, C, N], BF16)
        logits = st.tile([P, C, O], F32)
        bsel = st.tile([P, BL], BF16)
        brep = st.tile([BL, P], BF16)
        biasr = st.tile([BL, N], F32)

        nc.sync.dma_start(out=bsel[:], in_=bsel_d[:])
        nc.sync.dma_start(out=brep[:], in_=brep_d[:])
        nc.sync.dma_start(out=biasr[:], in_=bias_d[:])

        # ---- votes ----
        GRP = 4     # paired chunks per psum tile
        SLOT = 512  # psum bank-aligned slot
        NDMA = 8
        with tc.tile_pool(name="ph1", bufs=1) as ph1, tc.tile_pool(
            name="psv", bufs=2, space="PSUM"
        ) as psv:
            xbd = ph1.tile([P, CP, P], BF16)
            # split input DMAs so early matmuls can start sooner
            dstep = CP // NDMA
            for q in range(NDMA):
                sl = slice(q * dstep, (q + 1) * dstep)
                nc.sync.dma_start(out=xbd[:, sl, :], in_=xbd_d[:, sl, :])
                nc.sync.dma_start(out=w2c[:, sl, :], in_=w2c_d[:, sl, :])
            for g in range(CP // GRP):  # 24 groups
                ps = psv.tile([P, GRP * SLOT], F32, tag="pv")
                for j in range(GRP):
                    cp = g * GRP + j
                    nc.tensor.matmul(
                        ps[:, j * SLOT : j * SLOT + N2],
                        lhsT=xbd[:, cp, :],
                        rhs=w2c[:, cp, :],
                        start=True,
                        stop=True,
                    )
                src = ps[:].rearrange("p (j s) -> p j s", j=GRP)[:, :, 0:N2]
                dst = votes[:, g * 2 * GRP : (g + 1) * 2 * GRP, :].rearrange(
                    "p (j c2) n -> p j (c2 n)", j=GRP
                )
                if g % 2 == 0:
                    nc.scalar.copy(dst, src)
                else:
                    nc.vector.tensor_copy(dst, src)

        # ---- routing iterations ----
        pss = ctx.enter_context(tc.tile_pool(name="pss", bufs=1, space="PSUM"))
        SW = 3          # half-chunks per s-matmul
        NS = C // SW    # 48 s-matmuls per iteration

        for t in range(1, NUM_ROUTING + 1):
            if t == 1:
                svotes = votes
            else:
                expb = itp.tile([P, C, O], BF16, tag="expb")
                H = C // 2
                nc.scalar.activation(expb[:, 0:H], logits[:, 0:H], AF.Exp)
                nc.scalar.activation(expb[:, H:C], logits[:, H:C], AF.Exp)
                z = itp.tile([P, C], F32, tag="z")
                nc.vector.reduce_sum(z[:], expb[:], axis=AX.X)
                rz = itp.tile([P, C], F32, tag="rz")
                nc.vector.reciprocal_approx_fast(rz[:], z[:])
                route = itp.tile([P, C, O], BF16, tag="route")
                nc.vector.tensor_mul(
                    route[:], expb[:], rz[:].unsqueeze(2).broadcast_to([P, C, O])
                )
                wv = itp.tile([P, C, N], BF16, tag="big")
                v4 = votes[:].rearrange("p c (oa o) -> p c oa o", o=O)
                r4 = route[:].unsqueeze(2).broadcast_to([P, C, OA, O])
                w4 = wv[:].rearrange("p c (oa o) -> p c oa o", o=O)
                WPC = C // 4
                for wp in range(4):
                    sl = slice(wp * WPC, (wp + 1) * WPC)
                    nc.vector.tensor_mul(w4[:, sl], v4[:, sl], r4[:, sl])
                if CSPLIT < C:
                    nc.gpsimd.tensor_mul(
                        w4[:, CSPLIT:C], v4[:, CSPLIT:C], r4[:, CSPLIT:C]
                    )
                svotes = wv

            s_ps = pss.tile([BL, SW * N], F32, tag="sps")
            for j in range(NS):
                rhs = svotes[:, j * SW : (j + 1) * SW, :].rearrange("p c n -> p (c n)")
                nc.tensor.matmul(
                    s_ps[:], lhsT=bsel[:], rhs=rhs, start=(j == 0), stop=(j == NS - 1)
                )
            s3 = itp.tile([BL, SW, N], F32, tag="s3")
            nc.scalar.copy(s3[:], s_ps[:].rearrange("b (c n) -> b c n", c=SW))
            sa = itp.tile([BL, N], F32, tag="sa")
            nc.vector.tensor_add(sa[:], s3[:, 0, :], s3[:, 1, :])
            s_t = itp.tile([BL, N], F32, tag="stile")
            if t == 1:
                # s = (sa + s3[2]) * (1/O) + bias, folded:
                nc.vector.tensor_add(sa[:], sa[:], s3[:, 2, :])
                nc.vector.scalar_tensor_tensor(
                    s_t[:], sa[:], 1.0 / O, biasr[:], op0=ALU.mult, op1=ALU.add
                )
            else:
                nc.vector.tensor_add(sa[:], sa[:], s3[:, 2, :])
                nc.vector.tensor_add(s_t[:], sa[:], biasr[:])

            # squash along o: nsq[b, oa] = sum_o s^2
            sq = itp.tile([BL, N], F32, tag="sq")
            nc.vector.tensor_mul(sq[:], s_t[:], s_t[:])
            nsq = itp.tile([BL, OA], F32, tag="nsq")
            nc.vector.reduce_sum(
                nsq[:], sq[:].rearrange("b (oa o) -> b oa o", o=O), axis=AX.X
            )
            nsq1 = itp.tile([BL, OA], F32, tag="nsq1")
            nc.vector.tensor_scalar_add(nsq1[:], nsq[:], 1.0)
            rn1 = itp.tile([BL, OA], F32, tag="rn1")
            nc.vector.reciprocal_approx_fast(rn1[:], nsq1[:])
            sr = itp.tile([BL, OA], F32, tag="sr")
            nc.scalar.activation(sr[:], nsq[:], AF.Sqrt)
            f = itp.tile([BL, OA], F32, tag="f")
            nc.vector.tensor_mul(f[:], sr[:], rn1[:])
            vt = itp.tile([BL, N], F32, tag="vt")
            nc.vector.tensor_mul(
                vt[:].rearrange("b (oa o) -> b oa o", o=O),
                s_t[:].rearrange("b (oa o) -> b oa o", o=O),
                f[:].unsqueeze(2).broadcast_to([BL, OA, O]),
            )
            if t == NUM_ROUTING:
                nc.sync.dma_start(out=vout_d[:], in_=vt[:])
                break

            vbf = itp.tile([BL, N], BF16, tag="vbf")
            nc.vector.tensor_copy(vbf[:], vt[:])
            vr_ps = pss.tile([P, N], F32, tag="vrps")
            nc.tensor.matmul(vr_ps[:], lhsT=brep[:], rhs=vbf[:], start=True, stop=True)
            vrep = itp.tile([P, N], BF16, tag="vrep")
            nc.scalar.copy(vrep[:], vr_ps[:])

            tmp = itp.tile([P, C, N], BF16, tag="big")
            vr_b = vrep[:].unsqueeze(1).broadcast_to([P, C, N])
            nc.vector.tensor_mul(tmp[:, 0:CSPLIT], votes[:, 0:CSPLIT], vr_b[:, 0:CSPLIT])
            if CSPLIT < C:
                nc.gpsimd.tensor_mul(
                    tmp[:, CSPLIT:C], votes[:, CSPLIT:C], vr_b[:, CSPLIT:C]
                )
            t4 = tmp[:].rearrange("p c (oa o) -> p c oa o", o=O)
            for h in (8, 4, 2):
                nc.vector.tensor_add(
                    t4[:, 0:CSPLIT, 0:h, :],
                    t4[:, 0:CSPLIT, 0:h, :],
                    t4[:, 0:CSPLIT, h : 2 * h, :],
                )
                if CSPLIT < C:
                    nc.gpsimd.tensor_add(
                        t4[:, CSPLIT:C, 0:h, :],
                        t4[:, CSPLIT:C, 0:h, :],
                        t4[:, CSPLIT:C, h : 2 * h, :],
                    )
            if t == 1:
                H2 = C // 2
                for hs in (slice(0, H2), slice(H2, C)):
                    nc.vector.tensor_add(logits[:, hs], t4[:, hs, 0, :], t4[:, hs, 1, :])
            else:
                d = itp.tile([P, C, O], F32, tag="dd")
                H2 = C // 2
                for hs in (slice(0, H2), slice(H2, C)):
                    nc.vector.tensor_add(d[:, hs], t4[:, hs, 0, :], t4[:, hs, 1, :])
                    nc.vector.tensor_add(logits[:, hs], logits[:, hs], d[:, hs])

    nc.compile()
    return nc


def get_nc():
    if "nc" not in _NC_CACHE:
        _NC_CACHE["nc"] = _build_nc()
    return _NC_CACHE["nc"]


def make_in_maps(x, weights, biases):
    bf = ml_dtypes.bfloat16
    x = np.asarray(x, np.float32)
    weights = np.asarray(weights, np.float32)
    biases = np.asarray(biases, np.float32)

    # w2c[(h, is, a), cp, h2*N + (oa, o)] = w[(2cp+h)*8+is, a, o*16+oa] * (h==h2)
    w5 = (
        weights.reshape(CP, 2, IS8, A, O, OA)
        .transpose(0, 1, 2, 3, 5, 4)
        .reshape(CP, 2, IS8, A, N)
    )
    w2c = np.zeros((CP, 2, IS8, A, 2, N), np.float32)
    for h in range(2):
        w2c[:, h, :, :, h, :] = w5[:, h]
    w2c = w2c.reshape(CP, P, N2).transpose(1, 0, 2).astype(bf)

    eye = np.eye(BL, dtype=np.float32)
    bsel = np.tile(eye, (IS8, 1)).astype(bf)  # bsel[p, b'] = delta(p % BL == b')
    brep = np.tile(eye, (1, IS8)).astype(bf)  # brep[b, p] = delta(b == p % BL)
    biasr = np.broadcast_to(biases.T.reshape(1, N), (BL, N)).astype(np.float32).copy()

    in_maps = []
    idx = np.arange(IS8)
    for k in range(NCORES):
        xc = x[k * BL : (k + 1) * BL]  # [BL, I, A]
        xt = xc.reshape(BL, C, IS8, A).transpose(2, 1, 3, 0)  # [IS8, C, A, BL]
        xbd = np.zeros((C, IS8, A, IS8, BL), np.float32)
        # LHS advanced-index result shape: [IS8, C, A, BL]; RHS xt matches.
        xbd[:, idx, :, idx, :] = xt
        # [C=2*CP, (is,a)=64, (is',b)=128] -> pair chunks into k=128
        xbd = xbd.reshape(CP, 2 * IS8 * A, IS8 * BL).transpose(1, 0, 2).astype(bf)
        in_maps.append(
            {
                "xbd": np.ascontiguousarray(xbd),
                "w2c": w2c,
                "bsel": bsel,
                "brep": brep,
                "biasr": biasr,
            }
        )
    return in_maps


def assemble_out(results):
    out = np.zeros((B, 1, O, OA), np.float32)
    for k in range(NCORES):
        v = np.asarray(results[k]["vout"], np.float32)  # [BL, N], n = oa*O + o
        out[k * BL : (k + 1) * BL, 0] = v.reshape(BL, OA, O).transpose(0, 2, 1)
    return out


def kernel(x, weights, biases):
    from concourse.bass_utils import run_bass_kernel_spmd

    nc = get_nc()
    in_maps = make_in_maps(x, weights, biases)
    res = run_bass_kernel_spmd(nc, in_maps, list(range(NCORES)))
    return assemble_out(res.results)

